# revision 53
# baseline (speedup 1.0000x reference)
"""MHSA3D Trainium2 kernel v2: 8-way head-parallel attention with
PE-array-tiled qk, log2-domain softmax split across ScalarE + VectorE,
and 2-way col-tiled PV.

Problem (hardcoded): B=1, C=128, D=H=W=16 -> N=4096 tokens, 8 heads,
dh=16, dv=128.  One head per NeuronCore.

Key structure per core:
- Logits are produced in log2 domain (log2e folded into wq/bq on host),
  so exp(s) == 2^t.  ScalarE computes it as Exp with scale=ln2; VectorE
  computes it with a Schraudolph bit-trick (tensor_scalar -> int32
  convert, bitcast, cast to bf16), ~1.7% std error on its share.
- qk uses 4x4 PE array tiling: contraction is only dh=16 (padded to
  K=32), so 12 concurrent 32x32 tiles (3 row groups x 4 col strips)
  produce a [384 j, 512 i] logit batch in ~550ns instead of 4x 2130ns.
  q'/k' are replicated across 4 partition groups by the projection
  matmul itself (zero-padded weight columns).
- PSUM: stA/stB [128,1536] ping-pong batches (i-eighth pair), acc bank
  holds two PV accumulator strips (partitions 0-31 / 64-95) fed by
  2-way col-tiled PV matmuls (tile_position (0,0)/(0,64)), one spare
  bank for projection staging and the normalize broadcast.
- Denominator via an appended ones-column in vaug (M=32 stationary);
  v-bias folded into v before PV; normalize = reciprocal_approx_fast +
  ones-broadcast matmul + one tensor_tensor multiply.
"""

import numpy as np

NHEADS = 8
DV = 128
DH = DV // NHEADS  # 16
C = 128
N = 4096
IW = 512           # i-eighth width
NE = N // IW       # 8
NCHUNK = 32        # 128-wide j-chunks
LOG2E = 1.4426950408889634
LN2 = 0.6931471805599453
MAGIC = float(2 ** 23 * (127 - 0.058))
SCALE1 = float(2 ** 23)

# j-batches per eighth: 8x 512, each written as two 2-bank ring tiles
JB = [512 * k for k in range(8)]

# exp half-batches assigned to the DVE (Schraudolph), cycling pattern
# (1 = DVE, 0 = ScalarE).  All-zero disables the DVE path.
DVE_PAT = (0, 0, 1)
# keep-warm dummy matmuls emitted after each qk half-batch (HAM duty)
WARM_MMS = 1

_compiled = None


def _build_program():
    import concourse.bacc as bacc
    import concourse.mybir as mybir
    import concourse.tile as tile

    f32 = mybir.dt.float32
    f32r = mybir.dt.float32r
    bf16 = mybir.dt.bfloat16
    fp16 = mybir.dt.float16
    i32 = mybir.dt.int32
    EXP = mybir.ActivationFunctionType.Exp
    ADD = mybir.AluOpType.add
    MULT = mybir.AluOpType.mult

    nc = bacc.Bacc("TRN2", target_bir_lowering=False, debug=False,
                   num_devices=NHEADS)

    x_d = nc.dram_tensor("x", [C, N], fp16, kind="ExternalInput")
    wq4_d = nc.dram_tensor("wq4", [C, 128], fp16, kind="ExternalInput")
    wk4_d = nc.dram_tensor("wk4", [C, 128], fp16, kind="ExternalInput")
    wv_d = nc.dram_tensor("wv", [C, DH], fp16, kind="ExternalInput")
    bq4_d = nc.dram_tensor("bq4", [128, 1], f32, kind="ExternalInput")
    bk4_d = nc.dram_tensor("bk4", [128, N], bf16, kind="ExternalInput")
    bvp_d = nc.dram_tensor("bvp", [128, 512], bf16, kind="ExternalInput")
    vz_d = nc.dram_tensor("vz", [128, 32 * NCHUNK], f32r,
                          kind="ExternalInput")
    o_d = nc.dram_tensor("out", [DH, N], f32, kind="ExternalOutput")

    with tile.TileContext(nc) as tc:
        with (
            tc.tile_pool(name="const", bufs=1) as const,
            tc.tile_pool(name="pt", bufs=8) as ptp,
            tc.tile_pool(name="yi", bufs=3) as yip,
            tc.tile_pool(name="os", bufs=2) as osp,
            tc.tile_pool(name="ps", bufs=1, space="PSUM") as psp,
        ):
            x_s = const.tile([C, N], fp16)
            wq4_s = const.tile([C, 128], fp16)
            wk4_s = const.tile([C, 128], fp16)
            wv_s = const.tile([C, DH], fp16)
            bq4_s = const.tile([128, 1], f32)
            bk4_s = const.tile([128, N], bf16)
            bvp_s = const.tile([128, 512], bf16)
            qzc = [const.tile([128, 512], fp16, name=f"qzc{i}")
                   for i in range(8)]
            kzc = [const.tile([128, 512], fp16, name=f"kzc{i}")
                   for i in range(8)]
            vaugT = const.tile([128, 32 * NCHUNK], bf16)
            ones32 = const.tile([1, 32], f32)
            zerob = const.tile([128, 1], f32)
            scr1 = const.tile([128, 1], f32)

            st3 = [psp.tile([128, 1024], f32, name=f"st{i}")
                   for i in range(3)]
            accb = psp.tile([128, 512], f32)
            spare = psp.tile([128, 512], f32)

            nc.vector.memset(vaugT[:], 0.0)
            for q4 in range(4):
                q4s = slice(q4 * 1024, (q4 + 1) * 1024)
                nc.sync.dma_start(x_s[:, q4s], x_d.ap()[:, q4s])
            nc.scalar.dma_start(wq4_s[:], wq4_d.ap())
            nc.scalar.dma_start(wk4_s[:], wk4_d.ap())
            nc.scalar.dma_start(wv_s[:], wv_d.ap())
            nc.scalar.dma_start(bq4_s[:], bq4_d.ap())
            for q4 in range(4):
                q4s = slice(q4 * 1024, (q4 + 1) * 1024)
                nc.gpsimd.dma_start(bk4_s[:, q4s], bk4_d.ap()[:, q4s])
            nc.gpsimd.dma_start(bvp_s[:], bvp_d.ap())
            nc.gpsimd.memset(ones32[:], 1.0)
            nc.gpsimd.memset(zerob[:], 0.0)
            # warm the exp table set while DMAs run
            nc.scalar.activation(scr1[:], zerob[:], EXP, bias=zerob[:])
            # HAM warm-up: ~7us of back-to-back dummy matmuls during the
            # x DMA, so the PE enters the main pipeline unthrottled
            for _ in range(16):
                nc.tensor.matmul(spare[96:128, :],
                                 lhsT=bk4_s[0:32, 0:32],
                                 rhs=bk4_s[0:32, 0:512],
                                 start=True, stop=True,
                                 tile_position=(0, 96),
                                 skip_group_check=True)

            # ---- projections --------------------------------------
            # q/k chunks rotate through 4 one-bank psum slots
            slots = [accb[:, 0:512], spare[:, 0:512],
                     st3[0][:, 0:512], st3[1][:, 0:512]]
            slot_i = 0
            for i in range(8):
                sl = slots[slot_i % 4]; slot_i += 1
                nc.tensor.matmul(sl, lhsT=wk4_s[:], rhs=x_s[:, i * 512:(i + 1) * 512],
                                 start=True, stop=True)
                nc.vector.tensor_tensor(kzc[i][:], sl,
                                        bk4_s[:, i * 512:(i + 1) * 512], ADD)
            for i in range(8):
                sl = slots[slot_i % 4]; slot_i += 1
                nc.tensor.matmul(sl, lhsT=wq4_s[:], rhs=x_s[:, i * 512:(i + 1) * 512],
                                 start=True, stop=True)
                nc.vector.tensor_scalar_add(qzc[i][:], sl, bq4_s[:])

            # v^T chunks into accb, then fold bv and build vaug
            for m in range(NCHUNK):
                nc.tensor.matmul(accb[:, DH * m:DH * (m + 1)],
                                 lhsT=x_s[:, 128 * m:128 * (m + 1)],
                                 rhs=wv_s[:], start=True, stop=True)
            va3 = vaugT[:].rearrange("p (c s) -> p c s", s=32)
            vp3 = accb[:].rearrange("p (c s) -> p c s", s=DH)
            bv3 = bvp_s[:].rearrange("p (c s) -> p c s", s=DH)
            # vaug col 0 = ones (denominator row at a 32-aligned
            # partition); cols 1..16 = v + bv
            nc.vector.memset(va3[:, :, 0:1], 1.0)
            nc.vector.tensor_tensor(va3[:, :, 1:DH + 1], vp3[:, :, :],
                                    bv3[:, :, :], ADD)

            # ---- main loop ----------------------------------------
            from collections import deque
            pend_pv = deque()
            pending_tail_b = None

            def emit_qk_half(stx, e, jbase, half):
                # 2 M=128 row tiles (row groups 2h,2h+1): j256
                for b in range(2):
                    r = 2 * half + b
                    jslice = jbase + 128 * r
                    kt = kzc[jslice // 512]
                    off = jslice % 512
                    nc.tensor.matmul(
                        stx[:, 512 * b:512 * (b + 1)],
                        lhsT=kt[32 * r:32 * r + 32, off:off + 128],
                        rhs=qzc[e][32 * r:32 * r + 32, :],
                        start=True, stop=True,
                        tile_position=(32 * r, 0))

            exp_ctr = [0]

            def emit_exp(stx):
                pt = ptp.tile([128, 1024], bf16, tag="pt")
                i = exp_ctr[0]
                exp_ctr[0] += 1
                if DVE_PAT[i % len(DVE_PAT)]:
                    yi = yip.tile([128, 1024], i32, tag="yi")
                    nc.vector.tensor_scalar(yi[:], stx[:],
                                            SCALE1, MAGIC, MULT, ADD)
                    nc.vector.tensor_copy(pt[:], yi[:].bitcast(f32))
                else:
                    nc.scalar.activation(pt[:], stx[:], EXP,
                                         bias=zerob[:], scale=LN2)
                return pt

            def emit_warm():
                # dummy matmul into unread spare rows: pure PE activity to
                # keep the HAM duty cycle above the throttle threshold
                for _ in range(WARM_MMS):
                    nc.tensor.matmul(spare[96:128, :],
                                     lhsT=kzc[0][0:32, 0:32],
                                     rhs=qzc[0][0:32, :],
                                     start=True, stop=True,
                                     tile_position=(0, 96),
                                     skip_group_check=True)

            def make_pv(ptA, ptB, jbase):
                def emit():
                    for m in range(2):
                        jc = jbase // 128 + m
                        va = vaugT[:, 32 * jc:32 * jc + 32]
                        nc.tensor.matmul(
                            accb[0:32, :],
                            lhsT=va,
                            rhs=ptA[:, 512 * m:512 * (m + 1)],
                            start=(jc == 0), stop=(jc == NCHUNK - 1),
                            tile_position=(0, 0),
                            skip_group_check=True)
                        nc.tensor.matmul(
                            accb[64:96, :],
                            lhsT=va,
                            rhs=ptB[:, 512 * m:512 * (m + 1)],
                            start=(jc == 0), stop=(jc == NCHUNK - 1),
                            tile_position=(0, 64),
                            skip_group_check=True)
                return emit

            def make_tail_a(p):
                osb = osp.tile([128, 512], f32, tag="osb")
                rA = osp.tile([1, 512], f32, tag="rA")
                rB = osp.tile([1, 512], f32, tag="rB")

                dB = osp.tile([1, 512], f32, tag="dB")

                def emit():
                    nc.vector.tensor_copy(osb[:], accb[:])
                    # custom DVE ops cannot shift partitions; stage the B
                    # denominator to partition 0 with a stock copy first
                    nc.vector.tensor_copy(dB[:], osb[64:65, :])
                    nc.vector.reciprocal_approx_fast(rA[:], osb[0:1, :])
                    nc.vector.reciprocal_approx_fast(rB[:], dB[:])
                return emit, osb, rA, rB

            def make_tail_b(p, osb, rA, rB):
                def emit():
                    nc.tensor.matmul(spare[0:32, :], lhsT=ones32[:],
                                     rhs=rA[:], start=True, stop=True,
                                     tile_position=(0, 0),
                                     skip_group_check=True)
                    nc.tensor.matmul(spare[32:64, :], lhsT=ones32[:],
                                     rhs=rB[:], start=True, stop=True,
                                     tile_position=(0, 32),
                                     skip_group_check=True)
                    oA = osp.tile([32, 512], f32, tag="oA")
                    oB = osp.tile([32, 512], f32, tag="oB")
                    nc.vector.tensor_tensor(oA[:], osb[0:32, :],
                                            spare[0:32, :], MULT)
                    nc.vector.tensor_tensor(oB[:], osb[64:96, :],
                                            spare[32:64, :], MULT)
                    eA, eB = 2 * p, 2 * p + 1
                    nc.sync.dma_start(o_d.ap()[:, eA * IW:(eA + 1) * IW],
                                      oA[1:DH + 1, :])
                    nc.sync.dma_start(o_d.ap()[:, eB * IW:(eB + 1) * IW],
                                      oB[1:DH + 1, :])
                return emit

            ring = [0]
            for p in range(4):
                eA, eB = 2 * p, 2 * p + 1
                for jbi, jbase in enumerate(JB):
                    tA0 = st3[ring[0] % 3]
                    tA1 = st3[(ring[0] + 1) % 3]
                    tB0 = st3[(ring[0] + 2) % 3]
                    tB1 = st3[(ring[0] + 3) % 3]
                    ring[0] += 4
                    # drain old PV work FIRST so it sits ahead of this
                    # step's qk on the PE queue (avoids a cross-queue
                    # cycle via pt-buffer reuse)
                    while len(pend_pv) >= 4:
                        pend_pv.popleft()()
                    # tB1 aliases tA0 (ring of 3): its qk must be emitted
                    # AFTER exp(tA0) so Tile orders the overwrite correctly
                    emit_qk_half(tA0, eA, jbase, 0)
                    emit_warm()
                    emit_qk_half(tA1, eA, jbase, 1)
                    ptA0 = emit_exp(tA0)
                    emit_qk_half(tB0, eB, jbase, 0)
                    emit_warm()
                    ptA1 = emit_exp(tA1)
                    emit_qk_half(tB1, eB, jbase, 1)
                    emit_warm()
                    if pending_tail_b is not None and jbi == 2:
                        pending_tail_b()
                        pending_tail_b = None
                    ptB0 = emit_exp(tB0)
                    ptB1 = emit_exp(tB1)
                    pend_pv.append(make_pv(ptA0, ptB0, jbase))
                    pend_pv.append(make_pv(ptA1, ptB1, jbase + 256))
                while pend_pv:
                    pend_pv.popleft()()
                emit_a, osb, rA, rB = make_tail_a(p)
                emit_a()
                pending_tail_b = make_tail_b(p, osb, rA, rB)
            pending_tail_b()

    nc.compile()
    return nc


def _get_program():
    global _compiled
    if _compiled is None:
        _compiled = _build_program()
    return _compiled


def _to_bf16(x):
    import ml_dtypes
    return np.ascontiguousarray(
        np.asarray(x, np.float32)).astype(ml_dtypes.bfloat16)


def _prepare_core_inputs(x, w_qkv, b_qkv, emb_d, emb_h, emb_w):
    x2 = np.ascontiguousarray(
        np.asarray(x, np.float32).reshape(C, N)).astype(np.float16)
    w_qkv = np.asarray(w_qkv, np.float32)
    b_qkv = np.asarray(b_qkv, np.float32)
    qs = (DH ** -0.5) * LOG2E
    emb = (np.asarray(emb_d, np.float32)
           + np.asarray(emb_h, np.float32)
           + np.asarray(emb_w, np.float32)).reshape(DH, N)
    in_maps = []
    for h in range(NHEADS):
        qc = slice(h * DH, (h + 1) * DH)
        kc = slice(DV + h * DH, DV + (h + 1) * DH)
        vc = slice(2 * DV + h * DH, 2 * DV + (h + 1) * DH)
        wq4 = np.zeros((C, 128), np.float32)
        wk4 = np.zeros((C, 128), np.float32)
        bq4 = np.zeros((128, 1), np.float32)
        bk4 = np.zeros((128, N), np.float32)
        for r in range(4):
            wq4[:, 32 * r:32 * r + DH] = w_qkv[:, qc] * qs
            wk4[:, 32 * r:32 * r + DH] = w_qkv[:, kc]
            bq4[32 * r:32 * r + DH, 0] = b_qkv[qc] * qs
            bk4[32 * r:32 * r + DH, :] = b_qkv[kc][:, None] + emb
        bvp = np.tile(b_qkv[vc][None, :], (128, 32))
        vz = np.zeros((128, 1024), np.float32)
        vz[:, 0::32] = 1.0
        in_maps.append({
            "vz": vz,
            "x": x2,
            "wq4": wq4.astype(np.float16),
            "wk4": wk4.astype(np.float16),
            "wv": np.ascontiguousarray(w_qkv[:, vc]).astype(np.float16),
            "bq4": bq4,
            "bk4": _to_bf16(bk4),
            "bvp": _to_bf16(bvp),
        })
    return in_maps


def kernel(x, w_qkv, b_qkv, emb_d, emb_h, emb_w):
    from concourse.bass_utils import run_bass_kernel_spmd

    nc = _get_program()
    in_maps = _prepare_core_inputs(x, w_qkv, b_qkv, emb_d, emb_h, emb_w)
    res = run_bass_kernel_spmd(nc, in_maps, list(range(NHEADS)))
    out = np.empty((DV, N), np.float32)
    for h in range(NHEADS):
        out[h * DH:(h + 1) * DH, :] = res.results[h]["out"]
    return out.reshape(1, DV, 16, 16, 16)


# revision 54
# speedup vs baseline: 1.1327x; 1.1327x over previous
"""MHSA3D Trainium2 kernel v2: 8-way head-parallel attention with
PE-array-tiled qk, log2-domain softmax split across ScalarE + VectorE,
and 2-way col-tiled PV.

Problem (hardcoded): B=1, C=128, D=H=W=16 -> N=4096 tokens, 8 heads,
dh=16, dv=128.  One head per NeuronCore.

Key structure per core:
- Logits are produced in log2 domain (log2e folded into wq/bq on host),
  so exp(s) == 2^t.  ScalarE computes it as Exp with scale=ln2; VectorE
  computes it with a Schraudolph bit-trick (tensor_scalar -> int32
  convert, bitcast, cast to bf16), ~1.7% std error on its share.
- qk uses 4x4 PE array tiling: contraction is only dh=16 (padded to
  K=32), so 12 concurrent 32x32 tiles (3 row groups x 4 col strips)
  produce a [384 j, 512 i] logit batch in ~550ns instead of 4x 2130ns.
  q'/k' are replicated across 4 partition groups by the projection
  matmul itself (zero-padded weight columns).
- PSUM: stA/stB [128,1536] ping-pong batches (i-eighth pair), acc bank
  holds two PV accumulator strips (partitions 0-31 / 64-95) fed by
  2-way col-tiled PV matmuls (tile_position (0,0)/(0,64)), one spare
  bank for projection staging and the normalize broadcast.
- Denominator via an appended ones-column in vaug (M=32 stationary);
  v-bias folded into v before PV; normalize = reciprocal_approx_fast +
  ones-broadcast matmul + one tensor_tensor multiply.
"""

import numpy as np

NHEADS = 8
DV = 128
DH = DV // NHEADS  # 16
C = 128
N = 4096
IW = 512           # i-eighth width
NE = N // IW       # 8
NCHUNK = 32        # 128-wide j-chunks
LOG2E = 1.4426950408889634
LN2 = 0.6931471805599453
MAGIC = float(2 ** 23 * (127 - 0.058))
SCALE1 = float(2 ** 23)

# j-batches per eighth: 8x 512, each written as two 2-bank ring tiles
JB = [512 * k for k in range(8)]

# exp half-batches assigned to the DVE (Schraudolph), cycling pattern
# (1 = DVE, 0 = ScalarE).  All-zero disables the DVE path.
DVE_PAT = (0, 0, 1)
# keep-warm dummy matmuls emitted after each qk half-batch (HAM duty)
WARM_MMS = 1

_compiled = None


def _build_program():
    import concourse.bacc as bacc
    import concourse.mybir as mybir
    import concourse.tile as tile

    f32 = mybir.dt.float32
    f32r = mybir.dt.float32r
    bf16 = mybir.dt.bfloat16
    fp16 = mybir.dt.float16
    i32 = mybir.dt.int32
    EXP = mybir.ActivationFunctionType.Exp
    ADD = mybir.AluOpType.add
    MULT = mybir.AluOpType.mult

    nc = bacc.Bacc("TRN2", target_bir_lowering=False, debug=False,
                   num_devices=NHEADS)

    x_d = nc.dram_tensor("x", [C, N], fp16, kind="ExternalInput")
    wq4_d = nc.dram_tensor("wq4", [C, 128], fp16, kind="ExternalInput")
    wk4_d = nc.dram_tensor("wk4", [C, 128], fp16, kind="ExternalInput")
    wv_d = nc.dram_tensor("wv", [C, DH], fp16, kind="ExternalInput")
    bq4_d = nc.dram_tensor("bq4", [128, 1], f32, kind="ExternalInput")
    bk4_d = nc.dram_tensor("bk4", [128, N], bf16, kind="ExternalInput")
    bvp_d = nc.dram_tensor("bvp", [128, 512], bf16, kind="ExternalInput")
    vz_d = nc.dram_tensor("vz", [128, 32 * NCHUNK], f32r,
                          kind="ExternalInput")
    o_d = nc.dram_tensor("out", [DH, N], f32, kind="ExternalOutput")

    with tile.TileContext(nc) as tc:
        with (
            tc.tile_pool(name="const", bufs=1) as const,
            tc.tile_pool(name="pt", bufs=8) as ptp,
            tc.tile_pool(name="yi", bufs=3) as yip,
            tc.tile_pool(name="os", bufs=2) as osp,
            tc.tile_pool(name="ps", bufs=1, space="PSUM") as psp,
        ):
            x_s = const.tile([C, N], fp16)
            wq4_s = const.tile([C, 128], fp16)
            wk4_s = const.tile([C, 128], fp16)
            wv_s = const.tile([C, DH], fp16)
            bq4_s = const.tile([128, 1], f32)
            bk4_s = const.tile([128, N], bf16)
            bvp_s = const.tile([128, 512], bf16)
            qzc = [const.tile([128, 512], fp16, name=f"qzc{i}")
                   for i in range(8)]
            kzc = [const.tile([128, 512], fp16, name=f"kzc{i}")
                   for i in range(8)]
            vaugT = const.tile([128, 32 * NCHUNK], bf16)
            ones32 = const.tile([1, 32], f32)
            zerob = const.tile([128, 1], f32)
            scr1 = const.tile([128, 1], f32)

            st3 = [psp.tile([128, 1024], f32, name=f"st{i}")
                   for i in range(3)]
            accb = psp.tile([128, 512], f32)
            spare = psp.tile([128, 512], f32)

            nc.vector.memset(vaugT[:], 0.0)
            for q4 in range(4):
                q4s = slice(q4 * 1024, (q4 + 1) * 1024)
                nc.sync.dma_start(x_s[:, q4s], x_d.ap()[:, q4s])
            nc.scalar.dma_start(wq4_s[:], wq4_d.ap())
            nc.scalar.dma_start(wk4_s[:], wk4_d.ap())
            nc.scalar.dma_start(wv_s[:], wv_d.ap())
            nc.scalar.dma_start(bq4_s[:], bq4_d.ap())
            for q4 in range(4):
                q4s = slice(q4 * 1024, (q4 + 1) * 1024)
                nc.gpsimd.dma_start(bk4_s[:, q4s], bk4_d.ap()[:, q4s])
            nc.gpsimd.dma_start(bvp_s[:], bvp_d.ap())
            nc.gpsimd.memset(ones32[:], 1.0)
            nc.gpsimd.memset(zerob[:], 0.0)
            # warm the exp table set while DMAs run
            nc.scalar.activation(scr1[:], zerob[:], EXP, bias=zerob[:])
            # HAM warm-up: ~7us of back-to-back dummy matmuls during the
            # x DMA, so the PE enters the main pipeline unthrottled
            for _ in range(16):
                nc.tensor.matmul(spare[96:128, :],
                                 lhsT=bk4_s[0:32, 0:32],
                                 rhs=bk4_s[0:32, 0:512],
                                 start=True, stop=True,
                                 tile_position=(0, 96),
                                 skip_group_check=True)

            # ---- projections --------------------------------------
            # q/k chunks rotate through 4 one-bank psum slots
            slots = [accb[:, 0:512], spare[:, 0:512],
                     st3[0][:, 0:512], st3[1][:, 0:512]]
            slot_i = 0
            for i in range(8):
                sl = slots[slot_i % 4]; slot_i += 1
                nc.tensor.matmul(sl, lhsT=wk4_s[:], rhs=x_s[:, i * 512:(i + 1) * 512],
                                 start=True, stop=True)
                nc.vector.tensor_tensor(kzc[i][:], sl,
                                        bk4_s[:, i * 512:(i + 1) * 512], ADD)
            for i in range(8):
                sl = slots[slot_i % 4]; slot_i += 1
                nc.tensor.matmul(sl, lhsT=wq4_s[:], rhs=x_s[:, i * 512:(i + 1) * 512],
                                 start=True, stop=True)
                nc.vector.tensor_scalar_add(qzc[i][:], sl, bq4_s[:])

            # v^T chunks into accb, then fold bv and build vaug
            for m in range(NCHUNK):
                nc.tensor.matmul(accb[:, DH * m:DH * (m + 1)],
                                 lhsT=x_s[:, 128 * m:128 * (m + 1)],
                                 rhs=wv_s[:], start=True, stop=True)
            va3 = vaugT[:].rearrange("p (c s) -> p c s", s=32)
            vp3 = accb[:].rearrange("p (c s) -> p c s", s=DH)
            bv3 = bvp_s[:].rearrange("p (c s) -> p c s", s=DH)
            # vaug col 0 = ones (denominator row at a 32-aligned
            # partition); cols 1..16 = v + bv
            nc.vector.memset(va3[:, :, 0:1], 1.0)
            nc.vector.tensor_tensor(va3[:, :, 1:DH + 1], vp3[:, :, :],
                                    bv3[:, :, :], ADD)

            # ---- main loop ----------------------------------------
            from collections import deque
            pend_pv = deque()
            pending_tail_b = None

            def emit_qk_half(stx, e, jbase, half):
                # 8 tiles (row groups 2h,2h+1 x 4 col strips): j256
                for b in range(2):
                    r = 2 * half + b
                    for c in range(4):
                        jslice = jbase + 128 * r + 32 * c
                        kt = kzc[jslice // 512]
                        off = jslice % 512
                        nc.tensor.matmul(
                            stx[32 * c:32 * c + 32, 512 * b:512 * (b + 1)],
                            lhsT=kt[32 * r:32 * r + 32, off:off + 32],
                            rhs=qzc[e][32 * r:32 * r + 32, :],
                            start=True, stop=True,
                            tile_position=(32 * r, 32 * c))

            exp_ctr = [0]

            def emit_exp(stx):
                pt = ptp.tile([128, 1024], bf16, tag="pt")
                i = exp_ctr[0]
                exp_ctr[0] += 1
                if DVE_PAT[i % len(DVE_PAT)]:
                    yi = yip.tile([128, 1024], i32, tag="yi")
                    nc.vector.tensor_scalar(yi[:], stx[:],
                                            SCALE1, MAGIC, MULT, ADD)
                    nc.vector.tensor_copy(pt[:], yi[:].bitcast(f32))
                else:
                    nc.scalar.activation(pt[:], stx[:], EXP,
                                         bias=zerob[:], scale=LN2)
                return pt

            def emit_warm():
                # dummy matmul into unread spare rows: pure PE activity to
                # keep the HAM duty cycle above the throttle threshold
                for _ in range(WARM_MMS):
                    nc.tensor.matmul(spare[96:128, :],
                                     lhsT=kzc[0][0:32, 0:32],
                                     rhs=qzc[0][0:32, :],
                                     start=True, stop=True,
                                     tile_position=(0, 96),
                                     skip_group_check=True)

            def make_pv(ptA, ptB, jbase):
                def emit():
                    for m in range(2):
                        jc = jbase // 128 + m
                        va = vaugT[:, 32 * jc:32 * jc + 32]
                        nc.tensor.matmul(
                            accb[0:32, :],
                            lhsT=va,
                            rhs=ptA[:, 512 * m:512 * (m + 1)],
                            start=(jc == 0), stop=(jc == NCHUNK - 1),
                            tile_position=(0, 0),
                            skip_group_check=True)
                        nc.tensor.matmul(
                            accb[64:96, :],
                            lhsT=va,
                            rhs=ptB[:, 512 * m:512 * (m + 1)],
                            start=(jc == 0), stop=(jc == NCHUNK - 1),
                            tile_position=(0, 64),
                            skip_group_check=True)
                return emit

            def make_tail_a(p):
                osb = osp.tile([128, 512], f32, tag="osb")
                rA = osp.tile([1, 512], f32, tag="rA")
                rB = osp.tile([1, 512], f32, tag="rB")

                dB = osp.tile([1, 512], f32, tag="dB")

                def emit():
                    nc.vector.tensor_copy(osb[:], accb[:])
                    # custom DVE ops cannot shift partitions; stage the B
                    # denominator to partition 0 with a stock copy first
                    nc.vector.tensor_copy(dB[:], osb[64:65, :])
                    nc.vector.reciprocal_approx_fast(rA[:], osb[0:1, :])
                    nc.vector.reciprocal_approx_fast(rB[:], dB[:])
                return emit, osb, rA, rB

            def make_tail_b(p, osb, rA, rB):
                def emit():
                    nc.tensor.matmul(spare[0:32, :], lhsT=ones32[:],
                                     rhs=rA[:], start=True, stop=True,
                                     tile_position=(0, 0),
                                     skip_group_check=True)
                    nc.tensor.matmul(spare[32:64, :], lhsT=ones32[:],
                                     rhs=rB[:], start=True, stop=True,
                                     tile_position=(0, 32),
                                     skip_group_check=True)
                    oA = osp.tile([32, 512], f32, tag="oA")
                    oB = osp.tile([32, 512], f32, tag="oB")
                    nc.vector.tensor_tensor(oA[:], osb[0:32, :],
                                            spare[0:32, :], MULT)
                    nc.vector.tensor_tensor(oB[:], osb[64:96, :],
                                            spare[32:64, :], MULT)
                    eA, eB = 2 * p, 2 * p + 1
                    nc.sync.dma_start(o_d.ap()[:, eA * IW:(eA + 1) * IW],
                                      oA[1:DH + 1, :])
                    nc.sync.dma_start(o_d.ap()[:, eB * IW:(eB + 1) * IW],
                                      oB[1:DH + 1, :])
                return emit

            ring = [0]
            for p in range(4):
                eA, eB = 2 * p, 2 * p + 1
                for jbi, jbase in enumerate(JB):
                    tA0 = st3[ring[0] % 3]
                    tA1 = st3[(ring[0] + 1) % 3]
                    tB0 = st3[(ring[0] + 2) % 3]
                    tB1 = st3[(ring[0] + 3) % 3]
                    ring[0] += 4
                    # drain old PV work FIRST so it sits ahead of this
                    # step's qk on the PE queue (avoids a cross-queue
                    # cycle via pt-buffer reuse)
                    while len(pend_pv) >= 4:
                        pend_pv.popleft()()
                    # tB1 aliases tA0 (ring of 3): its qk must be emitted
                    # AFTER exp(tA0) so Tile orders the overwrite correctly
                    emit_qk_half(tA0, eA, jbase, 0)
                    emit_warm()
                    emit_qk_half(tA1, eA, jbase, 1)
                    ptA0 = emit_exp(tA0)
                    emit_qk_half(tB0, eB, jbase, 0)
                    emit_warm()
                    ptA1 = emit_exp(tA1)
                    emit_qk_half(tB1, eB, jbase, 1)
                    emit_warm()
                    if pending_tail_b is not None and jbi == 2:
                        pending_tail_b()
                        pending_tail_b = None
                    ptB0 = emit_exp(tB0)
                    ptB1 = emit_exp(tB1)
                    pend_pv.append(make_pv(ptA0, ptB0, jbase))
                    pend_pv.append(make_pv(ptA1, ptB1, jbase + 256))
                while pend_pv:
                    pend_pv.popleft()()
                emit_a, osb, rA, rB = make_tail_a(p)
                emit_a()
                pending_tail_b = make_tail_b(p, osb, rA, rB)
            pending_tail_b()

    nc.compile()
    return nc


def _get_program():
    global _compiled
    if _compiled is None:
        _compiled = _build_program()
    return _compiled


def _to_bf16(x):
    import ml_dtypes
    return np.ascontiguousarray(
        np.asarray(x, np.float32)).astype(ml_dtypes.bfloat16)


def _prepare_core_inputs(x, w_qkv, b_qkv, emb_d, emb_h, emb_w):
    x2 = np.ascontiguousarray(
        np.asarray(x, np.float32).reshape(C, N)).astype(np.float16)
    w_qkv = np.asarray(w_qkv, np.float32)
    b_qkv = np.asarray(b_qkv, np.float32)
    qs = (DH ** -0.5) * LOG2E
    emb = (np.asarray(emb_d, np.float32)
           + np.asarray(emb_h, np.float32)
           + np.asarray(emb_w, np.float32)).reshape(DH, N)
    in_maps = []
    for h in range(NHEADS):
        qc = slice(h * DH, (h + 1) * DH)
        kc = slice(DV + h * DH, DV + (h + 1) * DH)
        vc = slice(2 * DV + h * DH, 2 * DV + (h + 1) * DH)
        wq4 = np.zeros((C, 128), np.float32)
        wk4 = np.zeros((C, 128), np.float32)
        bq4 = np.zeros((128, 1), np.float32)
        bk4 = np.zeros((128, N), np.float32)
        for r in range(4):
            wq4[:, 32 * r:32 * r + DH] = w_qkv[:, qc] * qs
            wk4[:, 32 * r:32 * r + DH] = w_qkv[:, kc]
            bq4[32 * r:32 * r + DH, 0] = b_qkv[qc] * qs
            bk4[32 * r:32 * r + DH, :] = b_qkv[kc][:, None] + emb
        bvp = np.tile(b_qkv[vc][None, :], (128, 32))
        vz = np.zeros((128, 1024), np.float32)
        vz[:, 0::32] = 1.0
        in_maps.append({
            "vz": vz,
            "x": x2,
            "wq4": wq4.astype(np.float16),
            "wk4": wk4.astype(np.float16),
            "wv": np.ascontiguousarray(w_qkv[:, vc]).astype(np.float16),
            "bq4": bq4,
            "bk4": _to_bf16(bk4),
            "bvp": _to_bf16(bvp),
        })
    return in_maps


def kernel(x, w_qkv, b_qkv, emb_d, emb_h, emb_w):
    from concourse.bass_utils import run_bass_kernel_spmd

    nc = _get_program()
    in_maps = _prepare_core_inputs(x, w_qkv, b_qkv, emb_d, emb_h, emb_w)
    res = run_bass_kernel_spmd(nc, in_maps, list(range(NHEADS)))
    out = np.empty((DV, N), np.float32)
    for h in range(NHEADS):
        out[h * DH:(h + 1) * DH, :] = res.results[h]["out"]
    return out.reshape(1, DV, 16, 16, 16)
